# revision 69
# baseline (speedup 1.0000x reference)
"""MultiHeadCrossAttention Trainium2 kernel (8-core SPMD, query-parallel).

Sharding: core c handles batch b=c//4, query rows [1024*(c%4), +1024), all 8
heads.  Each core returns a disjoint [256, 1024] slice of out^T for its batch;
the host gather is a pure concat + transpose.

On-device layout is fully transposed ([channel, position]), matching the raw
[B, C, H, W] input layout, so no transposes are needed anywhere:
  q^T/k^T : [d, pos]   via  lhsT=W^T chunk [c,32|128], rhs=x^T chunk [c, pos]
  scores^T: [kpos, q]  via  lhsT=k^T [32,128] row-tiled 4x, rhs=q^T [32,512]
  exp     : ACT, PSUM->SBUF bf16, FD=1024 (the kernel's critical path:
            256 ACTIVATEs x ~1.0us is the ~260us floor)
  attn@v  : lhsT=[v|1] [128,33], rhs=p^T [128,512], col-tiled 2x (out
            partitions 0-32 / 64-96); the ones column yields softmax
            denominators in rows 32/96 of the accumulating matmuls
  norm    : denominators DMA-gathered into [128,64] tiles, approx-reciprocal,
            DMA back to a [1,NQ] row, gpsimd partition-broadcast, DVE mul
  final   : y^T accumulated in SBUF, one K=32 matmul + DVE add per head

Emission order software-pipelines head h+1's projections under head h's
attention so the ACT engine starts exp'ing within ~10us of kernel start.
All DVE/ACT ops keep in/out on identical partition ranges (walrus verifier
requirement); every cross-partition move rides on DMA or the PE.
"""

import numpy as np
import ml_dtypes

B, C, N, HEADS, D = 2, 256, 4096, 8, 32
NQ = 1024          # queries per core
NCORES = 8
CC = C // 128      # contraction chunks (2)

BF16 = ml_dtypes.bfloat16

_cached = {}
CFG = {"colattn": False, "scores4": True, "interleave": True, "norm": True, "debug": False,
       # exp engine schedule, cycled per pss tile: A=scalar ACT, D=vector
       # Schraudolph (gpsimd has no PSUM port so it can't exp)
       "exp_sched": "AADAADAD", "defer_drain": True}


class _RunNow(list):
    def append(self, fn):
        fn()


_runnow = _RunNow()

# bf16 Schraudolph exp2: bits16 = round(y*2^7 + (127-c)*2^7); bitcast -> ~2^y
# (max rel err 3.3%, mean 1.8%; common-mode error cancels in softmax, and the
# residual add dilutes what's left ~40x in the final output)
EXP2_A = 128.0
EXP2_B = (127.0 - 0.0434609) * 128.0
LN2 = float(np.log(2.0))


def _build_nc():
    import concourse.bass as bass
    import concourse.bacc as bacc
    import concourse.tile as tile
    import concourse.mybir as mybir
    from contextlib import ExitStack

    fp32 = mybir.dt.float32
    bf16 = mybir.dt.bfloat16
    i16 = mybir.dt.int16
    Exp = mybir.ActivationFunctionType.Exp
    Mult = mybir.AluOpType.mult
    Add = mybir.AluOpType.add

    nc = bacc.Bacc("TRN2", target_bir_lowering=False, debug=False,
                   num_devices=NCORES)

    src_d = nc.dram_tensor("src_bf", [C, N], bf16, kind="ExternalInput")
    tgt_d = nc.dram_tensor("tgt_bf", [C, NQ], bf16, kind="ExternalInput")
    # residual tgt^T in 4-head-stacked layout (partitions 32j = head 4g+j,
    # cols NQ*g..) so the normalize mul/add run as [128,NQ] DVE ops
    tgt4_d = nc.dram_tensor("tgt4", [128, 2 * NQ], fp32,
                            kind="ExternalInput")
    wq4_d = nc.dram_tensor("wq4", [C, HEADS * 128], bf16, kind="ExternalInput")
    wk_d = nc.dram_tensor("wkT", [C, C], bf16, kind="ExternalInput")
    wv_d = nc.dram_tensor("wvT", [C, C], bf16, kind="ExternalInput")
    # Wo^T in 4-head-stacked layout: wo4[32j:32j+32, 256g+d] = Wo.T row block
    # of head h=4g+j -> final proj is 2 K=128 matmuls per output chunk
    wo4_d = nc.dram_tensor("wo4", [128, 2 * C], bf16, kind="ExternalInput")
    y_d = nc.dram_tensor("yT", [C, NQ], fp32, kind="ExternalOutput")
    if CFG["debug"]:
        dbg_xw0_d = nc.dram_tensor("dbg_xw0", [32, NQ], fp32,
                                   kind="ExternalOutput")
        dbg_rbs0_d = nc.dram_tensor("dbg_rbs0", [32, NQ], fp32,
                                    kind="ExternalOutput")
        dbg_xf0_d = nc.dram_tensor("dbg_xf0", [32, NQ], fp32,
                                   kind="ExternalOutput")

    with tile.TileContext(nc) as tc, ExitStack() as ctx:
        konst = ctx.enter_context(tc.tile_pool(name="konst", bufs=1))
        work = ctx.enter_context(tc.tile_pool(name="work", bufs=1))
        p_pool = ctx.enter_context(tc.tile_pool(name="p", bufs=4))
        sm_pool = ctx.enter_context(tc.tile_pool(name="sm", bufs=2))
        xb_pool = ctx.enter_context(tc.tile_pool(name="xb", bufs=2))
        # PSUM budget (8 banks): ps tiles are [128,1024] (2 banks each) x3,
        # po [128,512] (1 bank) x1, pj [128,512] (1 bank) x1.  pj MUST have
        # its own bank: sharing the ps pool collapses the scores lookahead
        # (a feed tile steals a pss slot and re-exposes the exp latency);
        # sharing po serializes feeds against the attn@v accumulator.
        ps_pool = ctx.enter_context(tc.tile_pool(name="ps", bufs=3, space="PSUM"))
        po_pool = ctx.enter_context(tc.tile_pool(name="po", bufs=1, space="PSUM"))
        pj_pool = ctx.enter_context(tc.tile_pool(name="pj", bufs=1, space="PSUM"))

        # ---- load inputs (ordered so kproj/vproj deps land first; the DMA
        # stream is HBM-bound ~15us and overlaps the first attention rounds)
        src_sb = konst.tile([128, CC * N], bf16, tag="src")
        tgt_sb = konst.tile([128, CC * NQ], bf16, tag="tgt")
        tgt4_sb = konst.tile([128, 2 * NQ], fp32, tag="tgt4")
        wq4_sb = konst.tile([128, CC * HEADS * 128], bf16, tag="wq4")
        wk_sb = konst.tile([128, CC * C], bf16, tag="wk")
        wv_sb = konst.tile([128, CC * C], bf16, tag="wv")
        wo4_sb = konst.tile([128, 2 * C], bf16, tag="wo4")

        def dma_w(w_sb, w_d, eng=None):
            for cc in range(CC):
                (eng or nc.sync).dma_start(w_sb[:, cc * C:(cc + 1) * C],
                                           w_d.ap()[128 * cc:128 * (cc + 1), :])

        def dma_src_half(half, eng=None):
            for cc in range(CC):
                (eng or nc.sync).dma_start(
                    src_sb[:, cc * N + 2048 * half: cc * N + 2048 * (half + 1)],
                    src_d.ap()[128 * cc:128 * (cc + 1),
                               2048 * half:2048 * (half + 1)])

        dma_w(wk_sb, wk_d)
        dma_src_half(0)
        dma_w(wv_sb, wv_d)
        for cc in range(CC):
            nc.sync.dma_start(wq4_sb[:, cc * 1024:(cc + 1) * 1024],
                              wq4_d.ap()[128 * cc:128 * (cc + 1), :])
        for cc in range(CC):
            nc.sync.dma_start(tgt_sb[:, cc * NQ:(cc + 1) * NQ],
                              tgt_d.ap()[128 * cc:128 * (cc + 1), :])
        dma_src_half(1)
        nc.sync.dma_start(tgt4_sb[:], tgt4_d.ap()[:, :])
        nc.sync.dma_start(wo4_sb[:], wo4_d.ap()[:, :])

        # ---- persistent tiles ---------------------------------------------
        kT = [konst.tile([128, 1024], bf16, tag=f"kT{h}", name=f"kT{h}")
              for h in range(HEADS)]
        qT = [konst.tile([128, NQ], bf16, tag=f"qT{h}", name=f"qT{h}")
              for h in range(HEADS)]
        v_sb = konst.tile([128, HEADS * 33 * 32], bf16, tag="v")
        for h in range(HEADS):
            ones_ap = v_sb[:].rearrange("p (h k c) -> p h k c", h=HEADS, k=32)[
                :, h, :, 32:33]
            nc.gpsimd.memset(ones_ap, 1.0)
        # row 32 holds the per-qb softmax denominator row (drained together
        # with the 32 data rows in ONE DVE copy from po)
        xwh = [work.tile([33, NQ], fp32, tag=f"xw{h}", name=f"xw{h}")
               for h in range(HEADS)]
        # softmax denominators, one [32,64] tile per head-PAIR at partition
        # base 0 (custom-DVE ops corrupt at base!=0 on HW): tile[p, f] =
        # sums_flat[64p+f], flat = 1024*(h%2) + q; a/b = col-tile halves
        sums_a = [work.tile([32, 64], fp32, tag=f"sa{i}", name=f"sa{i}")
                  for i in range(4)]
        sums_b = [work.tile([32, 64], fp32, tag=f"sb{i}", name=f"sb{i}")
                  for i in range(4)]
        ssum_p = [work.tile([32, 64], fp32, tag=f"ss{i}", name=f"ss{i}")
                  for i in range(4)]
        rsum_p = [work.tile([32, 64], fp32, tag=f"rs{i}", name=f"rs{i}")
                  for i in range(4)]
        if not CFG["colattn"]:
            for i in range(4):
                nc.gpsimd.memset(sums_b[i][:], 0.0)
        yacc = [work.tile([128, NQ], fp32, tag=f"yacc{t}", name=f"yacc{t}")
                for t in range(CC)]
        # 4-head-stacked residual+attn outputs for the K=128 final proj,
        # plus stacked staging for the normalized weights and reciprocal
        # denominator broadcasts ([128,NQ] DVE ops instead of 8x [32,NQ])
        xf4 = [work.tile([128, NQ], bf16, tag=f"xf4_{g}", name=f"xf4_{g}")
               for g in range(2)]
        xst4 = [work.tile([128, NQ], fp32, tag=f"xst4_{g}", name=f"xst4_{g}")
                for g in range(2)]
        rbs4 = [work.tile([128, NQ], fp32, tag=f"rbs4_{g}", name=f"rbs4_{g}")
                for g in range(2)]

        v_done = set()
        exp_state = {"n": 0}
        # cross-unit work (po drains, recip, normalize staging) runs after
        # the NEXT unit's first exp so it never stalls the exp engines
        pending = []

        def flush_pending():
            for fn in pending:
                fn()
            pending.clear()

        def emit_exp(p_sb, ps_in, nm):
            """Drain one [128,1024] fp32 score tile (log2 units) to bf16
            p=2^y, on the engine picked by the exp schedule."""
            eng = CFG["exp_sched"][exp_state["n"] % len(CFG["exp_sched"])]
            exp_state["n"] += 1
            if eng == "A":
                nc.scalar.activation(p_sb[:], ps_in, Exp, scale=LN2)
            else:  # "D" (gpsimd cannot read PSUM)
                nc.vector.tensor_scalar(p_sb[:].bitcast(i16), ps_in,
                                        EXP2_A, EXP2_B, Mult, Add)

        def vproj2(kcs):
            """Project 2 kpos chunks into one pj tile (cols 0:256 / 256:512)
            so the single-bank pj pool turns over once per half-round, not
            twice."""
            kcs = [kc for kc in kcs if kc not in v_done]
            if not kcs:
                return
            v_done.update(kcs)
            ps = pj_pool.tile([128, 512], fp32, tag="pj",
                              name=f"psv{kcs[0]}")
            for i, kc in enumerate(kcs):
                for cc in range(CC):
                    nc.tensor.matmul(
                        ps[:, 256 * i:256 * i + 256],
                        lhsT=src_sb[:, cc * N + 128 * kc:
                                    cc * N + 128 * kc + 128],
                        rhs=wv_sb[:, cc * C:(cc + 1) * C],
                        start=(cc == 0), stop=(cc == CC - 1),
                        tile_position=(0, 0))
            def drain():
                for i, kc in enumerate(kcs):
                    dest = v_sb[:].rearrange("p (h k c) -> p h k c",
                                             h=HEADS, k=32)[:, :, kc, 0:32]
                    nc.vector.tensor_copy(dest, ps[:, 256 * i:256 * i + 256])
            return drain

        def kqproj_steps(h):
            # k^T folded: strip g (partitions 32g..) holds kpos block b=4jj+g
            # at cols [512jj, +512); kc for 128-col slice m: 16*(m//4)+4g+(m%4)
            # Each step is an (mm, drain) closure pair so the caller can emit
            # the DVE drain one half-round AFTER the matmuls: a drain emitted
            # right before an exp blocks the strict-FIFO DVE queue on the
            # still-running matmuls and stalls attn@v on the late exp.
            steps = []

            def k_step(jj):
                cell = {}

                def mm():
                    ps = pj_pool.tile([128, 512], fp32, tag="pj",
                                      name=f"psk{h}_{jj}")
                    for cc in range(CC):
                        for g in range(4):
                            blk = 4 * jj + g
                            nc.tensor.matmul(
                                ps[32 * g:32 * g + 32, 0:512],
                                lhsT=wk_sb[:, cc * C + 32 * h: cc * C + 32 * h + 32],
                                rhs=src_sb[:, cc * N + 512 * blk: cc * N + 512 * blk + 512],
                                start=(cc == 0), stop=(cc == CC - 1),
                                tile_position=(0, 32 * g))
                    cell["ps"] = ps

                def drain():
                    nc.vector.tensor_copy(
                        kT[h][:, 512 * jj:512 * jj + 512],
                        cell["ps"][:, 0:512])
                return mm, drain

            def q_step(qb):
                cell = {}

                def mm():
                    ps = pj_pool.tile([128, 512], fp32, tag="pj",
                                      name=f"psq{h}_{qb}")
                    for cc in range(CC):
                        nc.tensor.matmul(
                            ps[:, 0:512],
                            lhsT=wq4_sb[:, cc * 1024 + 128 * h: cc * 1024 + 128 * h + 128],
                            rhs=tgt_sb[:, cc * NQ + 512 * qb: cc * NQ + 512 * qb + 512],
                            start=(cc == 0), stop=(cc == CC - 1),
                            tile_position=(0, 0))
                    cell["ps"] = ps

                def drain():
                    nc.vector.tensor_copy(qT[h][:, 512 * qb:512 * qb + 512],
                                          cell["ps"][:, 0:512])
                return mm, drain

            for jj in range(2):
                steps.append(k_step(jj))
            for qb in range(NQ // 512):
                steps.append(q_step(qb))
            return steps

        def kqproj(h):
            for mm, drain in kqproj_steps(h):
                mm()
                drain()

        def attn_unit(h, qb, feed=()):
            feed = list(feed)
            """One (head, 512-query-block) attention unit: 16 half-rounds of
            one [128,1024] score tile (2 strip-chunks) each, software-
            pipelined with LOOKAHEAD so scores(r+1..r+2) are emitted BEFORE
            attn@v(r).  Without the lookahead the PE FIFO stalls at attn@v(r)
            waiting on exp(r) and the exp engines starve every round."""
            # po allocated lazily at the first attn@v: with bufs=1 the slot
            # must not be recycled before the previous unit's deferred drain
            # (emitted via flush_pending at idx 0) has been traced
            po = None
            hrounds = [((0, 1) if r % 2 == 0 else (2, 3), r // 2)
                       for r in range(16)]
            LOOK = 2
            pend = []
            defer = []
            for idx in range(len(hrounds) + LOOK):
                if idx < len(hrounds):
                    strips, m = hrounds[idx]
                    ps = ps_pool.tile([128, 1024], fp32, tag="ps",
                                      name=f"ps{h}_{qb}_{idx}")
                    for gi, g in enumerate(strips):
                        nc.tensor.matmul(
                            ps[:, 512 * gi:512 * gi + 512],
                            lhsT=kT[h][32 * g:32 * g + 32,
                                       128 * m:128 * m + 128],
                            rhs=qT[h][32 * g:32 * g + 32,
                                      512 * qb:512 * qb + 512],
                            start=True, stop=True,
                            tile_position=(32 * g, 0))
                    p_sb = p_pool.tile([128, 1024], bf16, tag="p",
                                       name=f"p{h}_{qb}_{idx}")
                    emit_exp(p_sb, ps[:, 0:1024], f"e{h}_{qb}_{idx}")
                    pend.append((strips, m, p_sb))
                    if idx == 0:
                        flush_pending()

                    # Drains (DVE) run one half-round after their matmuls so
                    # the strict-FIFO DVE queue never stalls ahead of an exp.
                    # INVARIANT (pj bufs=1): all outstanding pj drains must
                    # be emitted before the next pj-tile matmul recycles the
                    # slot, so flush `defer` before each pj use.
                    def use_pj(emitter):
                        for f in defer:
                            f()
                        defer.clear()
                        d = emitter()
                        if d:
                            defer.append(d)

                    # projection feeds go AFTER the scores group so they
                    # never delay the scores -> exp critical path; vproj for
                    # chunk kc is only consumed LOOK half-rounds later
                    if h == 0 and qb == 0:
                        kcs = [16 * (m // 4) + 4 * g + (m % 4)
                               for g in strips]
                        use_pj(lambda: vproj2(kcs))
                        if feed and idx % 2 == 1:
                            mm, dr = feed.pop(0)
                            use_pj(lambda: (mm(), dr)[1])
                    elif feed and idx % 2 == 0:
                        mm, dr = feed.pop(0)
                        use_pj(lambda: (mm(), dr)[1])
                if idx == len(hrounds):
                    # flush any drain backlog before the tail attn@v rounds
                    # (unit (0,0)'s vproj drains must precede their readers)
                    for dr in defer:
                        dr()
                    defer.clear()
                if idx >= LOOK:
                    if po is None:
                        po = po_pool.tile([128, 512], fp32, tag="po",
                                          name=f"po{h}_{qb}")
                    strips, m, p_sb = pend.pop(0)
                    ri = idx - LOOK
                    for gi, g in enumerate(strips):
                        kc = 16 * (m // 4) + 4 * g + (m % 4)
                        if CFG["colattn"]:
                            # the two chunks run concurrently in different
                            # col groups; halves merged at drain time
                            co = 64 * gi
                            st = ri == 0
                            sp = ri == len(hrounds) - 1
                        else:
                            co = 0
                            st = ri == 0 and gi == 0
                            sp = ri == len(hrounds) - 1 and gi == 1
                        nc.tensor.matmul(
                            po[co:co + 33, 0:512],
                            lhsT=v_sb[:, 1056 * h + 33 * kc:
                                      1056 * h + 33 * kc + 33],
                            rhs=p_sb[:, 512 * gi:512 * gi + 512],
                            start=st, stop=sp,
                            tile_position=(0, co))
            for mm, dr in feed:
                for f in defer:
                    f()
                defer.clear()
                mm()
                defer.append(dr)
            for dr in defer:
                dr()
            defer.clear()

            def drain_unit():
                # drain A (partitions 0-33, incl. denominator row 32) in one
                # DVE copy; B half (64-97) likewise, then DMA-hop + add
                hp, prow = h // 2, 16 * (h % 2) + 8 * qb
                nc.vector.tensor_copy(xwh[h][0:33, 512 * qb:512 * qb + 512],
                                      po[0:33, 0:512])
                nc.sync.dma_start(sums_a[hp][prow:prow + 8, 0:64],
                                  xwh[h][32:33, 512 * qb:512 * qb + 512])
                if CFG["colattn"]:
                    xb97 = xb_pool.tile([97, 512], fp32, tag="xb97",
                                        name=f"xb97_{h}{qb}")
                    nc.vector.tensor_copy(xb97[64:97, 0:512],
                                          po[64:97, 0:512])
                    xb0 = xb_pool.tile([32, 512], fp32, tag="xb0",
                                       name=f"xb0_{h}{qb}")
                    nc.sync.dma_start(xb0[:], xb97[64:96, 0:512])
                    nc.gpsimd.tensor_add(
                        xwh[h][0:32, 512 * qb:512 * qb + 512],
                        xwh[h][0:32, 512 * qb:512 * qb + 512], xb0[:])
                    nc.sync.dma_start(sums_b[hp][prow:prow + 8, 0:64],
                                      xb97[96:97, 0:512])
            if CFG["defer_drain"]:
                pending.append(drain_unit)
            else:
                drain_unit()


        def recip_pair(h):
            hp = h // 2
            nc.vector.tensor_add(ssum_p[hp][:], sums_a[hp][:], sums_b[hp][:])
            nc.vector.reciprocal_approx_fast(rsum_p[hp][:], ssum_p[hp][:])

        def normalize(h):
            """Stage head h's unnormalized weights and its broadcast
            1/denominator row into the 4-head-stacked tiles; the actual
            mul+add run once per group as [128,NQ] DVE ops."""
            g, j = h // 4, h % 4
            rrow = sm_pool.tile([1, NQ], fp32, tag="rrow", name=f"rr{h}")
            nc.sync.dma_start(rrow[:],
                              rsum_p[h // 2][16 * (h % 2):16 * (h % 2) + 16,
                                             0:64])
            rbs = sm_pool.tile([32, NQ], fp32, tag="rbs", name=f"rb{h}")
            nc.gpsimd.partition_broadcast(rbs[:], rrow[:])
            nc.sync.dma_start(rbs4[g][32 * j:32 * j + 32, :], rbs[:])
            nc.sync.dma_start(xst4[g][32 * j:32 * j + 32, :], xwh[h][0:32, :])

        def normalize_group(g):
            nc.vector.tensor_mul(xst4[g][:], xst4[g][:], rbs4[g][:])
            nc.vector.tensor_add(xf4[g][:], xst4[g][:],
                                 tgt4_sb[:, NQ * g:NQ * (g + 1)])

        def final_proj():
            """y^T = Wo^T (xf + tgt)^T with 4 heads stacked: K=128 matmuls,
            PSUM-accumulated over the two head groups."""
            for dc in range(CC):
                for qb in range(NQ // 512):
                    ps = pj_pool.tile([128, 512], fp32, tag="pj",
                                      name=f"py{dc}_{qb}")
                    for g in range(2):
                        nc.tensor.matmul(
                            ps[:, 0:512],
                            lhsT=wo4_sb[:, 256 * g + 128 * dc:
                                        256 * g + 128 * dc + 128],
                            rhs=xf4[g][:, 512 * qb:512 * qb + 512],
                            start=(g == 0), stop=(g == 1),
                            tile_position=(0, 0))
                    nc.vector.tensor_copy(
                        yacc[dc][:, 512 * qb:512 * qb + 512], ps[:, 0:512])

        # ---- emission: software-pipeline projections under attention ------
        if CFG["interleave"]:
            kqproj(0)
            for h in range(HEADS):
                steps = kqproj_steps(h + 1) if h + 1 < HEADS else []
                if h == 0:
                    attn_unit(h, 0, feed=steps[:2])
                    attn_unit(h, 1, feed=steps[2:])
                else:
                    attn_unit(h, 0, feed=steps)
                    attn_unit(h, 1)
                # normalization for pair p queues under pair p+1's attention
                # so its DMA/gpsimd latency never blocks the DVE queue
                q = pending if CFG["defer_drain"] else _runnow
                if h % 2 == 1:
                    q.append(lambda hh=h: recip_pair(hh))
                if h % 2 == 0 and h >= 2:
                    q.append(lambda hh=h: normalize(hh - 2))
                    q.append(lambda hh=h: normalize(hh - 1))
                    if h == 4:
                        q.append(lambda: normalize_group(0))
            flush_pending()
            normalize(HEADS - 2)
            normalize(HEADS - 1)
            normalize_group(1)
            final_proj()
        else:
            for kc in range(0, 32, 2):
                d = vproj2([kc, kc + 1])
                if d:
                    d()
            for h in range(HEADS):
                kqproj(h)
            for h in range(HEADS):
                for qb in range(NQ // 512):
                    attn_unit(h, qb)
                    flush_pending()
            for h in range(HEADS):
                if h % 2 == 1:
                    recip_pair(h)
                    normalize(h - 1)
                    normalize(h)
            normalize_group(0)
            normalize_group(1)
            final_proj()

        for dc in range(CC):
            nc.sync.dma_start(y_d.ap()[128 * dc:128 * (dc + 1), :],
                              yacc[dc][:])

    nc.compile()
    return nc


def _prep_core_inputs(core, tgt, src, Wq, Wk, Wv, Wo):
    b, qoff = core // 4, NQ * (core % 4)
    srcT = src[b].reshape(C, N)
    tgtT = tgt[b].reshape(C, N)[:, qoff:qoff + NQ]
    # log2(e) folded in: on-device scores are in log2 units (ACT applies
    # scale=ln2 inside the Exp activation; Schraudolph path wants 2^y)
    scale = np.float64(np.log2(np.e)) / np.sqrt(np.float64(D))
    wqT = (Wq * scale).T.astype(BF16)
    wq4 = np.empty((C, HEADS * 128), dtype=BF16)
    for h in range(HEADS):
        wq4[:, 128 * h:128 * (h + 1)] = np.tile(wqT[:, 32 * h:32 * h + 32],
                                                (1, 4))
    # residual tgt^T and Wo^T in 4-head-stacked layout (partitions 32j hold
    # head 4g+j's rows)
    tgt4 = np.empty((128, 2 * NQ), dtype=np.float32)
    woT = Wo.T.astype(np.float32)
    wo4 = np.empty((128, 2 * C), dtype=BF16)
    for h in range(HEADS):
        g, j = h // 4, h % 4
        tgt4[32 * j:32 * j + 32, NQ * g:NQ * (g + 1)] = \
            tgtT[32 * h:32 * h + 32, :]
        wo4[32 * j:32 * j + 32, C * g:C * (g + 1)] = \
            woT[32 * h:32 * h + 32, :].astype(BF16)
    return {
        "src_bf": np.ascontiguousarray(srcT).astype(BF16),
        "tgt_bf": np.ascontiguousarray(tgtT).astype(BF16),
        "tgt4": tgt4,
        "wq4": wq4,
        "wkT": np.ascontiguousarray(Wk.T).astype(BF16),
        "wvT": np.ascontiguousarray(Wv.T).astype(BF16),
        "wo4": wo4,
    }


def kernel(tgt, src, Wq, Wk, Wv, Wo, _want_results=False):
    from concourse.bass_utils import run_bass_kernel_spmd

    tgt = np.asarray(tgt, dtype=np.float32)
    src = np.asarray(src, dtype=np.float32)
    Wq = np.asarray(Wq, dtype=np.float32)
    Wk = np.asarray(Wk, dtype=np.float32)
    Wv = np.asarray(Wv, dtype=np.float32)
    Wo = np.asarray(Wo, dtype=np.float32)

    if "nc" not in _cached:
        _cached["nc"] = _build_nc()
    nc = _cached["nc"]

    in_maps = [_prep_core_inputs(c, tgt, src, Wq, Wk, Wv, Wo)
               for c in range(NCORES)]
    res = run_bass_kernel_spmd(nc, in_maps, core_ids=list(range(NCORES)))

    out = np.empty((B, N, C), dtype=np.float32)
    for c in range(NCORES):
        b, qoff = c // 4, NQ * (c % 4)
        out[b, qoff:qoff + NQ, :] = res.results[c]["yT"].T
    if _want_results:
        return out, res
    return out

